# revision 42
# baseline (speedup 1.0000x reference)
"""DigitCaps dynamic-routing kernel for 8 Trainium2 NeuronCores.

Problem: u_hat = einsum('rkoi,bri->brko', W[0], x); 3 routing iterations of
softmax-over-R / weighted-sum / squash / batch-mean agreement.
B=128, R=4608, K=2, O=32, I=16.

Strategy: shard R across the 8 cores (576 routes each).  u_hat (151 MB) is
NEVER materialized -- every routing contraction is pushed through the
factors x and W:

  s~[b,ko]  = sum_{r,i} x[b,r,i] * (exp(b_ij) ⊙ W)[r,i,ko]   (PE, psum-accum)
  a[r,k]    = sum_{i,o} Wt[r,i,ko] * G[r,i,ko],
  G[r,i,ko] = sum_b x[b,r,i] * v[b,ko]                        (PE)

Softmax normalization is deferred: each core AllGathers its partial s~
(32 KB) together with its local sum of exp(b_ij); every core then reduces
the 8 partials locally on the vector engine and normalizes inside squash.
b_ij updates (batch mean of agreement) are purely local to the r-shard.
"""

import sys

sys.path.insert(0, "/opt/trn_rl_repo")

import numpy as np

# Problem shapes (hardcoded; harness contract)
B, R, I, K, O = 128, 4608, 16, 2, 32
KO = K * O  # 64
NCORES = 8
RLOC = R // NCORES  # 576 routes per core
NG = RLOC // 8  # 72 groups of 8 routes (8r x 16i = 128 partitions)
NW = NG // 8  # 9 waves of 8 groups
NUM_ITER = 3

_PROGRAM = None  # cached (nc, names)


def _build_program(n_iter=NUM_ITER, enable_d=True):
    import concourse.bass as bass
    import concourse.tile as tile
    from concourse import bacc, mybir

    f32 = mybir.dt.float32
    bf16 = mybir.dt.bfloat16
    AF = mybir.ActivationFunctionType
    ALU = mybir.AluOpType

    nc = bacc.Bacc(
        "TRN2",
        target_bir_lowering=False,
        debug=False,
        num_devices=NCORES,
    )

    # ---------------- I/O ----------------
    xn_d = nc.dram_tensor("xn", [B, RLOC * I], bf16, kind="ExternalInput")
    xt_d = nc.dram_tensor("xt", [RLOC * I, B], bf16, kind="ExternalInput")
    wt_d = nc.dram_tensor("wt", [128, NG * KO], bf16, kind="ExternalInput")
    e88_d = nc.dram_tensor("e88", [128, 128], f32, kind="ExternalInput")
    ones1_d = nc.dram_tensor("ones1", [128, 1], f32, kind="ExternalInput")
    # final iteration outputs the PARTIAL s~ + local exp-sum; the host sums
    # the 8 partials and applies the last squash (part of the unshard step)
    vout_d = nc.dram_tensor("v_out", [B, KO + 2], f32, kind="ExternalOutput")

    # collective bounce buffers (one pair per iteration; payload = s~ [128,64]
    # in cols 0:64 plus the local exp-sum in row 0, cols 64:66)
    PAY = KO + 2  # 66
    cc_in = [
        nc.dram_tensor(f"cc_in{t}", [B, PAY], f32, kind="Internal")
        for t in range(NUM_ITER)
    ]
    cc_out = [
        nc.dram_tensor(
            f"cc_out{t}", [NCORES, B, PAY], f32, kind="Internal", addr_space="Shared"
        )
        for t in range(NUM_ITER)
    ]

    with tile.TileContext(nc) as tc:
        with (
            tc.tile_pool(name="persist", bufs=1) as persist,
            tc.tile_pool(name="work", bufs=2) as work,
            tc.tile_pool(name="stats", bufs=4) as stats,
            tc.tile_pool(name="psum_s", bufs=2, space="PSUM") as psum_s,
            tc.tile_pool(name="psum_g", bufs=3, space="PSUM") as psum_g,
            tc.tile_pool(name="psum_m", bufs=2, space="PSUM") as psum_m,
        ):
            # ------------- persistent SBUF state -------------
            xn_s = persist.tile([128, RLOC * I], bf16)  # [b | (g,j,i)]
            xt_s = persist.tile([128, NG, B], bf16)  # [(j,i) | g, b]
            wt_s = persist.tile([128, NG * KO], bf16)  # [(j,i) | (g,k,o)]
            cwt_s = persist.tile([128, NG * KO], bf16)  # c-scaled W
            e88_s = persist.tile([128, 128], f32)
            ones1_s = persist.tile([128, 1], f32)
            # b_ij, i-duplicated: [16j+i | (w, jj, k)]  (g = 8w+jj)
            b_idup = persist.tile([128, 2 * NG], f32)
            e_idup = persist.tile([128, 2 * NG], f32)
            zi0 = persist.tile([128, K], f32)
            i32 = mybir.dt.int32
            magic = persist.tile([128, K], i32)
            nc.vector.memset(magic[:], 0x5F3759DF)

            # ------------- load inputs -------------
            xt_view = xt_d.ap().rearrange("(g p) b -> p g b", p=128)
            for c in range(2):
                f0, f1 = 2304 * c, 2304 * (c + 1)
                nc.sync.dma_start(out=wt_s[:, f0:f1], in_=wt_d.ap()[:, f0:f1])
            for c in range(6):
                g0, g1 = 12 * c, 12 * (c + 1)
                nc.sync.dma_start(out=xt_s[:, g0:g1, :], in_=xt_view[:, g0:g1, :])
            nc.sync.dma_start(out=e88_s[:], in_=e88_d.ap())
            nc.sync.dma_start(out=ones1_s[:], in_=ones1_d.ap())
            for c in range(4):
                f0, f1 = 2304 * c, 2304 * (c + 1)
                nc.sync.dma_start(out=xn_s[:, f0:f1], in_=xn_d.ap()[:, f0:f1])
            nc.vector.memset(b_idup[:], 0.0)
            nc.vector.memset(zi0[:], 1.0 / R)

            # it0 (b)-pass: c uniform -> use wt directly (1/R folded in zi0)
            st_ps = psum_s.tile([128, KO], f32, tag="stilde")
            for g in range(NG):
                nc.tensor.matmul(
                    st_ps[:],
                    lhsT=xt_s[:, g, :],
                    rhs=wt_s[:, KO * g : KO * (g + 1)],
                    start=(g == 0),
                    stop=(g == NG - 1),
                )

            for it in range(n_iter):
                if it == n_iter - 1:
                    # final iteration: ship partial s~ + local Z to the host
                    vo = work.tile([128, PAY], f32, tag="payload")
                    nc.scalar.activation(
                        out=vo[:, 0:KO], in_=st_ps[:], func=AF.Copy
                    )
                    if it == 0:
                        nc.vector.memset(vo[:, KO : KO + 2], float(R) / NCORES)
                    else:
                        zpart = stats.tile([128, K], f32, tag="zpart")
                        nc.vector.tensor_reduce(
                            out=zpart[:],
                            in_=e_idup[:].rearrange(
                                "p (w jj k) -> p k (w jj)", w=NW, jj=8, k=K
                            ),
                            axis=mybir.AxisListType.X,
                            op=ALU.add,
                        )
                        zps = psum_m.tile([1, K], f32, tag="aix")
                        nc.tensor.matmul(
                            zps[:],
                            lhsT=ones1_s[:],
                            rhs=zpart[:],
                            start=True,
                            stop=True,
                        )
                        nc.vector.tensor_scalar_mul(
                            vo[0:1, KO : KO + 2], zps[:], 1.0 / 16.0
                        )
                    nc.sync.dma_start(out=vout_d.ap(), in_=vo[:])
                    continue

                # ------------- payload & AllGather -------------
                payload = work.tile([128, PAY], f32, tag="payload")
                nc.scalar.activation(
                    out=payload[:, 0:KO], in_=st_ps[:], func=AF.Copy
                )
                if it == 0:
                    nc.vector.memset(payload[:, KO : KO + 2], 0.0)
                else:
                    # local Z_k = sum_r exp(b) = (sum_p sum_{w,jj} e_idup)/16
                    zpart = stats.tile([128, K], f32, tag="zpart")
                    nc.vector.tensor_reduce(
                        out=zpart[:],
                        in_=e_idup[:].rearrange(
                            "p (w jj k) -> p k (w jj)", w=NW, jj=8, k=K
                        ),
                        axis=mybir.AxisListType.X,
                        op=ALU.add,
                    )
                    zps = psum_m.tile([1, K], f32, tag="aix")
                    nc.tensor.matmul(
                        zps[:], lhsT=ones1_s[:], rhs=zpart[:], start=True, stop=True
                    )
                    nc.vector.tensor_scalar_mul(
                        payload[0:1, KO : KO + 2], zps[:], 1.0 / 16.0
                    )
                nc.sync.dma_start(out=cc_in[it].ap(), in_=payload[:])
                nc.gpsimd.collective_compute(
                    "AllGather",
                    ALU.bypass,
                    replica_groups=[list(range(NCORES))],
                    ins=[cc_in[it].ap()],
                    outs=[cc_out[it].ap()],
                )

                # gather partial sums back and reduce over ranks
                sall = work.tile([128, NCORES, KO], f32, tag="sall")
                src = bass.AP(
                    tensor=cc_out[it],
                    offset=0,
                    ap=[[PAY, 128], [PAY * B, NCORES], [1, KO]],
                )
                nc.sync.dma_start(out=sall[:], in_=src)
                ssum = work.tile([128, KO], f32, tag="ssum")
                nc.vector.tensor_reduce(
                    out=ssum[:],
                    in_=sall[:].rearrange("b r f -> b f r"),
                    axis=mybir.AxisListType.X,
                    op=ALU.add,
                )
                if it == 0:
                    zi = zi0
                else:
                    zall = stats.tile([128, NCORES, K], f32, tag="zall")
                    zsrc = bass.AP(
                        tensor=cc_out[it],
                        offset=KO,
                        ap=[[0, 128], [PAY * B, NCORES], [1, K]],
                    )
                    nc.gpsimd.dma_start(out=zall[:], in_=zsrc)
                    zsum = stats.tile([128, K], f32, tag="zsum")
                    nc.vector.tensor_reduce(
                        out=zsum[:],
                        in_=zall[:].rearrange("b r f -> b f r"),
                        axis=mybir.AxisListType.X,
                        op=ALU.add,
                    )
                    zi = stats.tile([128, K], f32, tag="zi")
                    nc.vector.reciprocal(out=zi[:], in_=zsum[:])

                # ------------- squash -------------
                # sn = (sum_o s~^2) * zi^2 ; v = s~ * zi * sqrt(sn)/(0.5+sn)
                ssq = stats.tile([128, K], f32, tag="ssq")
                s2 = work.tile([128, KO], f32, tag="s2")
                nc.vector.tensor_mul(s2[:], ssum[:], ssum[:])
                nc.vector.tensor_reduce(
                    out=ssq[:],
                    in_=s2[:].rearrange("p (k o) -> p k o", k=K),
                    axis=mybir.AxisListType.X,
                    op=ALU.add,
                )
                # rsqrt(ssq) via bit-trick seed + 2 Newton steps -- runs on
                # ssq directly so it does NOT wait for the (slow) zi path;
                # zi^2 is folded in at the end.  sqrt(sn)/(0.5+sn)*zi
                #   = sqrt(ssq)*zi^2 / (0.5 + ssq*zi^2)
                ybits = stats.tile([128, K], i32, tag="ybits")
                nc.vector.tensor_scalar(
                    ybits[:], ssq[:].bitcast(i32), 1, None, ALU.arith_shift_right
                )
                nc.vector.tensor_sub(ybits[:], magic[:], ybits[:])
                y = ybits[:].bitcast(f32)
                t1 = stats.tile([128, K], f32, tag="t1")
                t2 = stats.tile([128, K], f32, tag="t2")
                for _ in range(2):
                    nc.vector.tensor_mul(t1[:], y, y)
                    nc.vector.tensor_mul(t1[:], t1[:], ssq[:])
                    nc.vector.tensor_scalar(
                        t2[:], t1[:], -0.5, 1.5, ALU.mult, ALU.add
                    )
                    nc.vector.tensor_mul(ybits[:].bitcast(f32), y, t2[:])
                sqs = stats.tile([128, K], f32, tag="sqs")
                nc.vector.tensor_mul(sqs[:], ssq[:], y)  # sqrt(ssq)
                zi2 = stats.tile([128, K], f32, tag="zi2")
                nc.vector.tensor_mul(zi2[:], zi[:], zi[:])
                sn2 = stats.tile([128, K], f32, tag="sn2")
                nc.vector.tensor_mul(sn2[:], ssq[:], zi2[:])
                den = stats.tile([128, K], f32, tag="den")
                nc.vector.tensor_scalar_add(den[:], sn2[:], 0.5)
                rden = stats.tile([128, K], f32, tag="rden")
                nc.vector.reciprocal(out=rden[:], in_=den[:])
                gfac = stats.tile([128, K], f32, tag="gfac")
                nc.vector.tensor_mul(gfac[:], sqs[:], zi2[:])
                nc.vector.tensor_mul(gfac[:], gfac[:], rden[:])

                if not enable_d:
                    continue
                # v in bf16 directly (only the G-matmuls consume it)
                v_bf = work.tile([128, KO], bf16, tag="v_bf")
                for k in range(K):
                    nc.vector.tensor_scalar_mul(
                        v_bf[:, O * k : O * (k + 1)],
                        ssum[:, O * k : O * (k + 1)],
                        gfac[:, k : k + 1],
                    )

                # ---- (d) agreement + b-update + prep of next (b), per wave ----
                st_next = psum_s.tile([128, KO], f32, tag="stilde")
                for w in range(NW):
                    wsl = slice(512 * w, 512 * (w + 1))
                    g_ps = psum_g.tile([128, 512], f32, tag="g_ps")
                    for jj in range(8):
                        g = 8 * w + jj
                        nc.tensor.matmul(
                            g_ps[:, KO * jj : KO * (jj + 1)],
                            lhsT=xn_s[:, 128 * g : 128 * (g + 1)],
                            rhs=v_bf[:],
                            start=True,
                            stop=True,
                        )
                    g_sb = work.tile([128, 512], bf16, tag="g_sb")
                    nc.scalar.activation(out=g_sb[:], in_=g_ps[:], func=AF.Copy)
                    pprod = work.tile([128, 512], bf16, tag="pprod")
                    nc.vector.tensor_mul(pprod[:], g_sb[:], wt_s[:, wsl])
                    a1w = stats.tile([128, 16], f32, tag="a1w")
                    nc.vector.tensor_reduce(
                        out=a1w[:].rearrange("p (jj k) -> p jj k", jj=8),
                        in_=pprod[:].rearrange("p (jj k o) -> p jj k o", jj=8, k=K),
                        axis=mybir.AxisListType.X,
                        op=ALU.add,
                    )
                    aix_ps = psum_m.tile([128, 16], f32, tag="aix")
                    nc.tensor.matmul(
                        aix_ps[:], lhsT=e88_s[:], rhs=a1w[:], start=True, stop=True
                    )
                    bsl = b_idup[:, 16 * w : 16 * w + 16]
                    nc.vector.scalar_tensor_tensor(
                        out=bsl,
                        in0=aix_ps[:],
                        scalar=1.0 / B,
                        in1=bsl,
                        op0=ALU.mult,
                        op1=ALU.add,
                    )
                    e_sl = e_idup[:, 16 * w : 16 * w + 16]
                    nc.scalar.activation(out=e_sl, in_=bsl, func=AF.Exp)
                    e_ap = bass.AP(
                        tensor=e_sl.tensor,
                        offset=e_sl.offset,
                        ap=[list(e_sl.ap[0]), [2, 8], [1, 2], [0, O]],
                    )
                    nc.gpsimd.tensor_tensor(
                        out=cwt_s[:, wsl].rearrange("p (jj k o) -> p jj k o", jj=8, k=K),
                        in0=wt_s[:, wsl].rearrange("p (jj k o) -> p jj k o", jj=8, k=K),
                        in1=e_ap,
                        op=ALU.mult,
                    )
                    for jj in range(8):
                        g = 8 * w + jj
                        nc.tensor.matmul(
                            st_next[:],
                            lhsT=xt_s[:, g, :],
                            rhs=cwt_s[:, KO * g : KO * (g + 1)],
                            start=(g == 0),
                            stop=(g == NG - 1),
                        )
                st_ps = st_next

    nc.compile()
    return nc


def _get_program():
    global _PROGRAM
    if _PROGRAM is None:
        import os

        n_iter = int(os.environ.get("KERNEL_N_ITER", str(NUM_ITER)))
        enable_d = os.environ.get("KERNEL_ENABLE_D", "1") == "1"
        _PROGRAM = _build_program(n_iter, enable_d)
    return _PROGRAM


def _prep_inputs(x, W):
    import ml_dtypes

    bf = ml_dtypes.bfloat16
    x = np.asarray(x, dtype=np.float32)
    W = np.asarray(W, dtype=np.float32)
    e88 = np.kron(np.eye(8, dtype=np.float32), np.ones((16, 16), dtype=np.float32))
    ones1 = np.ones((128, 1), dtype=np.float32)
    in_maps = []
    for c in range(NCORES):
        rs, re = c * RLOC, (c + 1) * RLOC
        xs = x[:, rs:re, :].astype(bf)  # [B, RLOC, I]
        xn = np.ascontiguousarray(xs.reshape(B, RLOC * I))
        xt = np.ascontiguousarray(xs.transpose(1, 2, 0).reshape(RLOC * I, B))
        Wl = W[0, rs:re].astype(bf)  # [RLOC, K, O, I]
        wt = np.ascontiguousarray(
            Wl.reshape(NG, 8, K, O, I).transpose(1, 4, 0, 2, 3).reshape(128, NG * KO)
        )
        in_maps.append({"xn": xn, "xt": xt, "wt": wt, "e88": e88, "ones1": ones1})
    return in_maps


def run(x, W, trace=False):
    from concourse import bass_utils

    nc = _get_program()
    in_maps = _prep_inputs(x, W)
    res = bass_utils.run_bass_kernel_spmd(
        nc, in_maps, core_ids=list(range(NCORES)), trace=trace
    )
    # unshard: sum the per-core partial s~ / Z, then the final squash
    parts = [np.asarray(res.results[c]["v_out"], np.float32) for c in range(NCORES)]
    tot = np.sum(parts, axis=0)  # [B, KO+2]
    z = tot[0, KO : KO + 2]  # [K]
    s = tot[:, :KO].reshape(B, K, O) / z[None, :, None]
    sn = (s * s).sum(-1, keepdims=True)
    v = sn * s / ((0.5 + sn) * np.sqrt(sn))
    return v.astype(np.float32), res


def kernel(x, W):
    v, _ = run(x, W, trace=False)
    return v


# revision 44
# speedup vs baseline: 1.1176x; 1.1176x over previous
"""DigitCaps dynamic-routing kernel for 8 Trainium2 NeuronCores.

Problem: u_hat = einsum('rkoi,bri->brko', W[0], x); 3 routing iterations of
softmax-over-R / weighted-sum / squash / batch-mean agreement.
B=128, R=4608, K=2, O=32, I=16.

Strategy: shard R across the 8 cores (576 routes each).  u_hat (151 MB) is
NEVER materialized -- every routing contraction is pushed through the
factors x and W:

  s~[b,ko]  = sum_{r,i} x[b,r,i] * (exp(b_ij) ⊙ W)[r,i,ko]   (PE, psum-accum)
  a[r,k]    = sum_{i,o} Wt[r,i,ko] * G[r,i,ko],
  G[r,i,ko] = sum_b x[b,r,i] * v[b,ko]                        (PE)

Softmax normalization is deferred: each core AllGathers its partial s~
(32 KB) together with its local sum of exp(b_ij); every core then reduces
the 8 partials locally on the vector engine and normalizes inside squash.
b_ij updates (batch mean of agreement) are purely local to the r-shard.
"""

import sys

sys.path.insert(0, "/opt/trn_rl_repo")

import numpy as np

# Problem shapes (hardcoded; harness contract)
B, R, I, K, O = 128, 4608, 16, 2, 32
KO = K * O  # 64
NCORES = 8
RLOC = R // NCORES  # 576 routes per core
NG = RLOC // 8  # 72 groups of 8 routes (8r x 16i = 128 partitions)
NW = NG // 8  # 9 waves of 8 groups
NUM_ITER = 3

_PROGRAM = None  # cached (nc, names)


def _build_program(n_iter=NUM_ITER, enable_d=True):
    import concourse.bass as bass
    import concourse.tile as tile
    from concourse import bacc, mybir

    f32 = mybir.dt.float32
    bf16 = mybir.dt.bfloat16
    AF = mybir.ActivationFunctionType
    ALU = mybir.AluOpType

    nc = bacc.Bacc(
        "TRN2",
        target_bir_lowering=False,
        debug=False,
        num_devices=NCORES,
    )

    # ---------------- I/O ----------------
    xn_d = nc.dram_tensor("xn", [B, RLOC * I], bf16, kind="ExternalInput")
    xt_d = nc.dram_tensor("xt", [RLOC * I, B], bf16, kind="ExternalInput")
    wt_d = nc.dram_tensor("wt", [128, NG * KO], bf16, kind="ExternalInput")
    e88_d = nc.dram_tensor("e88", [128, 128], f32, kind="ExternalInput")
    ones1_d = nc.dram_tensor("ones1", [128, 1], f32, kind="ExternalInput")
    # final iteration outputs the PARTIAL s~ + local exp-sum; the host sums
    # the 8 partials and applies the last squash (part of the unshard step)
    vout_d = nc.dram_tensor("v_out", [B, KO + 2], f32, kind="ExternalOutput")

    # collective bounce buffers (one pair per iteration; payload = s~ [128,64]
    # in cols 0:64 plus the local exp-sum in row 0, cols 64:66)
    PAY = KO + 2  # 66
    cc_in = [
        nc.dram_tensor(f"cc_in{t}", [B, PAY], f32, kind="Internal")
        for t in range(NUM_ITER)
    ]
    cc_out = [
        nc.dram_tensor(
            f"cc_out{t}", [NCORES, B, PAY], f32, kind="Internal", addr_space="Shared"
        )
        for t in range(NUM_ITER)
    ]

    with tile.TileContext(nc) as tc:
        with (
            tc.tile_pool(name="persist", bufs=1) as persist,
            tc.tile_pool(name="work", bufs=3) as work,
            tc.tile_pool(name="stats", bufs=4) as stats,
            tc.tile_pool(name="psum_s", bufs=2, space="PSUM") as psum_s,
            tc.tile_pool(name="psum_g", bufs=3, space="PSUM") as psum_g,
            tc.tile_pool(name="psum_m", bufs=3, space="PSUM") as psum_m,
        ):
            # ------------- persistent SBUF state -------------
            xn_s = persist.tile([128, RLOC * I], bf16)  # [b | (g,j,i)]
            xt_s = persist.tile([128, NG, B], bf16)  # [(j,i) | g, b]
            wt_s = persist.tile([128, NG * KO], bf16)  # [(j,i) | (g,k,o)]
            cwt_s = persist.tile([128, NG * KO], bf16)  # c-scaled W
            e88_s = persist.tile([128, 128], f32)
            ones1_s = persist.tile([128, 1], f32)
            # b_ij, i-duplicated: [16j+i | (w, jj, k)]  (g = 8w+jj)
            b_idup = persist.tile([128, 2 * NG], f32)
            e_idup = persist.tile([128, 2 * NG], f32)
            zi0 = persist.tile([128, K], f32)
            i32 = mybir.dt.int32
            magic = persist.tile([128, K], i32)
            nc.vector.memset(magic[:], 0x5F3759DF)

            # ------------- load inputs -------------
            xt_view = xt_d.ap().rearrange("(g p) b -> p g b", p=128)
            for c in range(2):
                f0, f1 = 2304 * c, 2304 * (c + 1)
                nc.sync.dma_start(out=wt_s[:, f0:f1], in_=wt_d.ap()[:, f0:f1])
            for c in range(6):
                g0, g1 = 12 * c, 12 * (c + 1)
                nc.sync.dma_start(out=xt_s[:, g0:g1, :], in_=xt_view[:, g0:g1, :])
            nc.sync.dma_start(out=e88_s[:], in_=e88_d.ap())
            nc.sync.dma_start(out=ones1_s[:], in_=ones1_d.ap())
            for c in range(4):
                f0, f1 = 2304 * c, 2304 * (c + 1)
                nc.sync.dma_start(out=xn_s[:, f0:f1], in_=xn_d.ap()[:, f0:f1])
            nc.vector.memset(b_idup[:], 0.0)
            nc.vector.memset(zi0[:], 1.0 / R)

            # it0 (b)-pass: c uniform -> use wt directly (1/R folded in zi0)
            st_ps = psum_s.tile([128, KO], f32, tag="stilde")
            for g in range(NG):
                nc.tensor.matmul(
                    st_ps[:],
                    lhsT=xt_s[:, g, :],
                    rhs=wt_s[:, KO * g : KO * (g + 1)],
                    start=(g == 0),
                    stop=(g == NG - 1),
                )

            for it in range(n_iter):
                if it == n_iter - 1:
                    # final iteration: ship partial s~ + local Z to the host
                    vo = work.tile([128, PAY], f32, tag="payload")
                    nc.scalar.activation(
                        out=vo[:, 0:KO], in_=st_ps[:], func=AF.Copy
                    )
                    if it == 0:
                        nc.vector.memset(vo[:, KO : KO + 2], float(R) / NCORES)
                    else:
                        zpart = stats.tile([128, K], f32, tag="zpart")
                        nc.vector.tensor_reduce(
                            out=zpart[:],
                            in_=e_idup[:].rearrange(
                                "p (w jj k) -> p k (w jj)", w=NW, jj=8, k=K
                            ),
                            axis=mybir.AxisListType.X,
                            op=ALU.add,
                        )
                        zps = psum_m.tile([1, K], f32, tag="aix")
                        nc.tensor.matmul(
                            zps[:],
                            lhsT=ones1_s[:],
                            rhs=zpart[:],
                            start=True,
                            stop=True,
                        )
                        nc.vector.tensor_scalar_mul(
                            vo[0:1, KO : KO + 2], zps[:], 1.0 / 16.0
                        )
                    nc.sync.dma_start(out=vout_d.ap(), in_=vo[:])
                    continue

                # ------------- payload & AllGather -------------
                payload = work.tile([128, PAY], f32, tag="payload")
                nc.scalar.activation(
                    out=payload[:, 0:KO], in_=st_ps[:], func=AF.Copy
                )
                if it == 0:
                    nc.vector.memset(payload[:, KO : KO + 2], 0.0)
                else:
                    # local Z_k = sum_r exp(b) = (sum_p sum_{w,jj} e_idup)/16
                    zpart = stats.tile([128, K], f32, tag="zpart")
                    nc.vector.tensor_reduce(
                        out=zpart[:],
                        in_=e_idup[:].rearrange(
                            "p (w jj k) -> p k (w jj)", w=NW, jj=8, k=K
                        ),
                        axis=mybir.AxisListType.X,
                        op=ALU.add,
                    )
                    zps = psum_m.tile([1, K], f32, tag="aix")
                    nc.tensor.matmul(
                        zps[:], lhsT=ones1_s[:], rhs=zpart[:], start=True, stop=True
                    )
                    nc.vector.tensor_scalar_mul(
                        payload[0:1, KO : KO + 2], zps[:], 1.0 / 16.0
                    )
                nc.sync.dma_start(out=cc_in[it].ap(), in_=payload[:])
                nc.gpsimd.collective_compute(
                    "AllGather",
                    ALU.bypass,
                    replica_groups=[list(range(NCORES))],
                    ins=[cc_in[it].ap()],
                    outs=[cc_out[it].ap()],
                )

                # gather partial sums back and reduce over ranks
                sall = work.tile([128, NCORES, KO], f32, tag="sall")
                src = bass.AP(
                    tensor=cc_out[it],
                    offset=0,
                    ap=[[PAY, 128], [PAY * B, NCORES], [1, KO]],
                )
                nc.sync.dma_start(out=sall[:], in_=src)
                ssum = work.tile([128, KO], f32, tag="ssum")
                nc.vector.tensor_reduce(
                    out=ssum[:],
                    in_=sall[:].rearrange("b r f -> b f r"),
                    axis=mybir.AxisListType.X,
                    op=ALU.add,
                )
                if it == 0:
                    zi = zi0
                else:
                    zall = stats.tile([128, NCORES, K], f32, tag="zall")
                    zsrc = bass.AP(
                        tensor=cc_out[it],
                        offset=KO,
                        ap=[[0, 128], [PAY * B, NCORES], [1, K]],
                    )
                    nc.gpsimd.dma_start(out=zall[:], in_=zsrc)
                    zsum = stats.tile([128, K], f32, tag="zsum")
                    nc.vector.tensor_reduce(
                        out=zsum[:],
                        in_=zall[:].rearrange("b r f -> b f r"),
                        axis=mybir.AxisListType.X,
                        op=ALU.add,
                    )
                    zi = stats.tile([128, K], f32, tag="zi")
                    nc.vector.reciprocal(out=zi[:], in_=zsum[:])

                # ------------- squash -------------
                # sn = (sum_o s~^2) * zi^2 ; v = s~ * zi * sqrt(sn)/(0.5+sn)
                ssq = stats.tile([128, K], f32, tag="ssq")
                s2 = work.tile([128, KO], f32, tag="s2")
                nc.vector.tensor_mul(s2[:], ssum[:], ssum[:])
                nc.vector.tensor_reduce(
                    out=ssq[:],
                    in_=s2[:].rearrange("p (k o) -> p k o", k=K),
                    axis=mybir.AxisListType.X,
                    op=ALU.add,
                )
                # rsqrt(ssq) via bit-trick seed + 2 Newton steps -- runs on
                # ssq directly so it does NOT wait for the (slow) zi path;
                # zi^2 is folded in at the end.  sqrt(sn)/(0.5+sn)*zi
                #   = sqrt(ssq)*zi^2 / (0.5 + ssq*zi^2)
                ybits = stats.tile([128, K], i32, tag="ybits")
                nc.vector.tensor_scalar(
                    ybits[:], ssq[:].bitcast(i32), 1, None, ALU.arith_shift_right
                )
                nc.vector.tensor_sub(ybits[:], magic[:], ybits[:])
                y = ybits[:].bitcast(f32)
                t1 = stats.tile([128, K], f32, tag="t1")
                t2 = stats.tile([128, K], f32, tag="t2")
                for _ in range(2):
                    nc.vector.tensor_mul(t1[:], y, y)
                    nc.vector.tensor_mul(t1[:], t1[:], ssq[:])
                    nc.vector.tensor_scalar(
                        t2[:], t1[:], -0.5, 1.5, ALU.mult, ALU.add
                    )
                    nc.vector.tensor_mul(ybits[:].bitcast(f32), y, t2[:])
                sqs = stats.tile([128, K], f32, tag="sqs")
                nc.vector.tensor_mul(sqs[:], ssq[:], y)  # sqrt(ssq)
                zi2 = stats.tile([128, K], f32, tag="zi2")
                nc.vector.tensor_mul(zi2[:], zi[:], zi[:])
                sn2 = stats.tile([128, K], f32, tag="sn2")
                nc.vector.tensor_mul(sn2[:], ssq[:], zi2[:])
                den = stats.tile([128, K], f32, tag="den")
                nc.vector.tensor_scalar_add(den[:], sn2[:], 0.5)
                rden = stats.tile([128, K], f32, tag="rden")
                nc.vector.reciprocal(out=rden[:], in_=den[:])
                gfac = stats.tile([128, K], f32, tag="gfac")
                nc.vector.tensor_mul(gfac[:], sqs[:], zi2[:])
                nc.vector.tensor_mul(gfac[:], gfac[:], rden[:])

                if not enable_d:
                    continue
                # v in bf16 directly (only the G-matmuls consume it)
                v_bf = work.tile([128, KO], bf16, tag="v_bf")
                for k in range(K):
                    nc.vector.tensor_scalar_mul(
                        v_bf[:, O * k : O * (k + 1)],
                        ssum[:, O * k : O * (k + 1)],
                        gfac[:, k : k + 1],
                    )

                # ---- (d) agreement + b-update + prep of next (b), per wave ----
                st_next = psum_s.tile([128, KO], f32, tag="stilde")
                for w in range(NW):
                    wsl = slice(512 * w, 512 * (w + 1))
                    g_ps = psum_g.tile([128, 512], f32, tag="g_ps")
                    for jj in range(8):
                        g = 8 * w + jj
                        nc.tensor.matmul(
                            g_ps[:, KO * jj : KO * (jj + 1)],
                            lhsT=xn_s[:, 128 * g : 128 * (g + 1)],
                            rhs=v_bf[:],
                            start=True,
                            stop=True,
                        )
                    g_sb = work.tile([128, 512], bf16, tag="g_sb")
                    nc.scalar.activation(out=g_sb[:], in_=g_ps[:], func=AF.Copy)
                    pprod = work.tile([128, 512], bf16, tag="pprod")
                    nc.vector.tensor_mul(pprod[:], g_sb[:], wt_s[:, wsl])
                    a1w = stats.tile([128, 16], f32, tag="a1w")
                    nc.vector.tensor_reduce(
                        out=a1w[:].rearrange("p (jj k) -> p jj k", jj=8),
                        in_=pprod[:].rearrange("p (jj k o) -> p jj k o", jj=8, k=K),
                        axis=mybir.AxisListType.X,
                        op=ALU.add,
                    )
                    aix_ps = psum_m.tile([128, 16], f32, tag="aix")
                    nc.tensor.matmul(
                        aix_ps[:], lhsT=e88_s[:], rhs=a1w[:], start=True, stop=True
                    )
                    bsl = b_idup[:, 16 * w : 16 * w + 16]
                    nc.vector.scalar_tensor_tensor(
                        out=bsl,
                        in0=aix_ps[:],
                        scalar=1.0 / B,
                        in1=bsl,
                        op0=ALU.mult,
                        op1=ALU.add,
                    )
                    e_sl = e_idup[:, 16 * w : 16 * w + 16]
                    nc.scalar.activation(out=e_sl, in_=bsl, func=AF.Exp)
                    e_ap = bass.AP(
                        tensor=e_sl.tensor,
                        offset=e_sl.offset,
                        ap=[list(e_sl.ap[0]), [2, 8], [1, 2], [0, O]],
                    )
                    nc.gpsimd.tensor_tensor(
                        out=cwt_s[:, wsl].rearrange("p (jj k o) -> p jj k o", jj=8, k=K),
                        in0=wt_s[:, wsl].rearrange("p (jj k o) -> p jj k o", jj=8, k=K),
                        in1=e_ap,
                        op=ALU.mult,
                    )
                    for jj in range(8):
                        g = 8 * w + jj
                        nc.tensor.matmul(
                            st_next[:],
                            lhsT=xt_s[:, g, :],
                            rhs=cwt_s[:, KO * g : KO * (g + 1)],
                            start=(g == 0),
                            stop=(g == NG - 1),
                        )
                st_ps = st_next

    nc.compile()
    return nc


def _get_program():
    global _PROGRAM
    if _PROGRAM is None:
        import os

        n_iter = int(os.environ.get("KERNEL_N_ITER", str(NUM_ITER)))
        enable_d = os.environ.get("KERNEL_ENABLE_D", "1") == "1"
        _PROGRAM = _build_program(n_iter, enable_d)
    return _PROGRAM


def _prep_inputs(x, W):
    import ml_dtypes

    bf = ml_dtypes.bfloat16
    x = np.asarray(x, dtype=np.float32)
    W = np.asarray(W, dtype=np.float32)
    e88 = np.kron(np.eye(8, dtype=np.float32), np.ones((16, 16), dtype=np.float32))
    ones1 = np.ones((128, 1), dtype=np.float32)
    in_maps = []
    for c in range(NCORES):
        rs, re = c * RLOC, (c + 1) * RLOC
        xs = x[:, rs:re, :].astype(bf)  # [B, RLOC, I]
        xn = np.ascontiguousarray(xs.reshape(B, RLOC * I))
        xt = np.ascontiguousarray(xs.transpose(1, 2, 0).reshape(RLOC * I, B))
        Wl = W[0, rs:re].astype(bf)  # [RLOC, K, O, I]
        wt = np.ascontiguousarray(
            Wl.reshape(NG, 8, K, O, I).transpose(1, 4, 0, 2, 3).reshape(128, NG * KO)
        )
        in_maps.append({"xn": xn, "xt": xt, "wt": wt, "e88": e88, "ones1": ones1})
    return in_maps


def run(x, W, trace=False):
    from concourse import bass_utils

    nc = _get_program()
    in_maps = _prep_inputs(x, W)
    res = bass_utils.run_bass_kernel_spmd(
        nc, in_maps, core_ids=list(range(NCORES)), trace=trace
    )
    # unshard: sum the per-core partial s~ / Z, then the final squash
    parts = [np.asarray(res.results[c]["v_out"], np.float32) for c in range(NCORES)]
    tot = np.sum(parts, axis=0)  # [B, KO+2]
    z = tot[0, KO : KO + 2]  # [K]
    s = tot[:, :KO].reshape(B, K, O) / z[None, :, None]
    sn = (s * s).sum(-1, keepdims=True)
    v = sn * s / ((0.5 + sn) * np.sqrt(sn))
    return v.astype(np.float32), res


def kernel(x, W):
    v, _ = run(x, W, trace=False)
    return v
